# revision 7
# baseline (speedup 1.0000x reference)
"""Trainium2 Bass kernel for the EI-RNN problem (nn_RNN_EI_13460427506020).

Reference computation (B=128, IN=1024, N=4096, T=48, DECAY=0.9):
    drive = x @ Wi + b0
    xm_t  = DECAY*xm_{t-1} + drive + f_{t-1} @ Wr ;  f_t = relu(xm_t)
    returns (stack of f_t, stack of xm_t) as [B, T, N] each.

Distribution: the hidden dimension N is column-sharded over the 8 cores
(NSH=512 columns each) so the 4096x512 Wr shard stays SBUF-resident in
bf16.  Each step every core computes its r-shard with full-batch (M=128)
matmuls over 32 k-tiles of the *transposed* firing rates fT, then the
cores all-gather their freshly produced fT shards (bf16, 128KB/core) for
the next step.  Matmul operands are bf16; state/accumulation are fp32.
"""

import os

import numpy as np

import concourse.bacc as bacc
import concourse.bass as bass
import concourse.mybir as mybir
import concourse.tile as tile
from concourse import masks
from concourse.bass_utils import run_bass_kernel_spmd

B, IN, N, T = 128, 1024, 4096, 48
DECAY = 0.9
NCORES = 8
NSH = N // NCORES          # 512 hidden columns per core
KT = 128                   # contraction tile
NKT = N // KT              # 32 k-tiles of the gathered fT
NKI = IN // KT             # 8 k-tiles of x
BF = mybir.dt.bfloat16
F32 = mybir.dt.float32
AF = mybir.ActivationFunctionType
ALU = mybir.AluOpType


def build_module(t_steps=T, passes=1):
    nc = bacc.Bacc(
        "TRN2",
        target_bir_lowering=False,
        debug=False,
        enable_asserts=False,
        num_devices=NCORES,
    )
    x_d = nc.dram_tensor("x", [B, IN], F32, kind="ExternalInput")
    wi_d = nc.dram_tensor("wi", [IN, NSH], F32, kind="ExternalInput")
    wr_d = nc.dram_tensor("wr", [N, NSH], F32, kind="ExternalInput")
    b0_d = nc.dram_tensor("b0", [1, NSH], F32, kind="ExternalInput")
    outf_d = nc.dram_tensor("outf", [B, t_steps, NSH], F32, kind="ExternalOutput")
    outu_d = nc.dram_tensor("outu", [B, t_steps, NSH], F32, kind="ExternalOutput")

    with tile.TileContext(nc) as tc:
        _kernel(tc, x_d, wi_d, wr_d, b0_d, outf_d, outu_d, t_steps, passes)
    nc.compile()
    return nc


def _kernel(tc, x_d, wi_d, wr_d, b0_d, outf_d, outu_d, t_steps, passes=1):
    nc = tc.nc
    with (
        tc.tile_pool(name="const", bufs=1) as cpool,
        tc.tile_pool(name="wr", bufs=1) as wrpool,
        tc.tile_pool(name="ld", bufs=4) as ldpool,
        tc.tile_pool(name="state", bufs=2) as stpool,
        tc.tile_pool(name="gather", bufs=2) as gpool,
        tc.tile_pool(name="send", bufs=2) as sndpool,
        tc.tile_pool(name="psum", bufs=2, space="PSUM") as pspool,
        tc.tile_pool(name="psumt", bufs=2, space="PSUM") as ptpool,
        tc.tile_pool(name="dram_in", bufs=2, space="DRAM") as dinpool,
        tc.tile_pool(name="dram_out", bufs=2, space="DRAM") as doutpool,
    ):
        ident_f = cpool.tile([128, 128], F32, tag="idf")
        masks.make_identity(nc, ident_f[:])
        ident_b = cpool.tile([128, 128], BF, tag="idb")
        masks.make_identity(nc, ident_b[:])

        # ---- load Wr shard, cast to bf16 k-tiles [128, NSH] each
        wr_b = wrpool.tile([128, NKT * NSH], BF, tag="wrb")
        for kt in range(NKT):
            tmp = ldpool.tile([128, NSH], F32, tag="ldtmp")
            nc.sync.dma_start(tmp[:], wr_d.ap()[KT * kt : KT * (kt + 1), :])
            nc.vector.tensor_copy(wr_b[:, NSH * kt : NSH * (kt + 1)], tmp[:])

        # ---- load x, transpose to xT (bf16 k-tiles [128, 128])
        x_sb = cpool.tile([128, IN], F32, tag="xsb")
        nc.sync.dma_start(x_sb[:], x_d.ap())
        xt_b = cpool.tile([128, NKI * 128], BF, tag="xtb")
        for k in range(NKI):
            pst = ptpool.tile([128, 128], F32, tag="pst_x")
            nc.tensor.transpose(pst[:], x_sb[:, KT * k : KT * (k + 1)], ident_f[:])
            nc.vector.tensor_copy(xt_b[:, 128 * k : 128 * (k + 1)], pst[:])

        # ---- load Wi shard, cast to bf16
        wi_b = cpool.tile([128, NKI * NSH], BF, tag="wib")
        for k in range(NKI):
            tmp = ldpool.tile([128, NSH], F32, tag="ldtmp")
            nc.sync.dma_start(tmp[:], wi_d.ap()[KT * k : KT * (k + 1), :])
            nc.vector.tensor_copy(wi_b[:, NSH * k : NSH * (k + 1)], tmp[:])

        # ---- b0 row + ones row for the bias matmul
        b0_b = cpool.tile([1, NSH], BF, tag="b0b")
        b0_f = cpool.tile([1, NSH], F32, tag="b0f")
        nc.sync.dma_start(b0_f[:], b0_d.ap())
        nc.vector.tensor_copy(b0_b[:], b0_f[:])
        ones = cpool.tile([1, 128], BF, tag="ones")
        nc.vector.memset(ones[:], 1.0)

        # ---- drive = x @ Wi + b0  (psum, fp32), then keep bf16 copy
        ps_drive = pspool.tile([128, NSH], F32, tag="ps")
        for k in range(NKI):
            nc.tensor.matmul(
                ps_drive[:],
                xt_b[:, 128 * k : 128 * (k + 1)],
                wi_b[:, NSH * k : NSH * (k + 1)],
                start=(k == 0),
                stop=False,
            )
        nc.tensor.matmul(ps_drive[:], ones[:], b0_b[:], start=False, stop=True)
        drive_b = cpool.tile([128, NSH], BF, tag="driveb")
        nc.vector.tensor_copy(drive_b[:], ps_drive[:])
        drive_f = cpool.tile([128, NSH], F32, tag="drivef")
        nc.scalar.activation(drive_f[:], ps_drive[:], AF.Copy)

        # ---- recurrent scan (repeated `passes` times for benchmarking)
        for _pass in range(passes):
            _scan_pass(tc, locals())


def _scan_pass(tc, env):
    nc = tc.nc
    (t_steps, stpool, pspool, ptpool, sndpool, gpool, dinpool, doutpool,
     drive_f, drive_b, wr_b, ident_f, ident_b, outf_d, outu_d) = (
        env[k] for k in (
            "t_steps", "stpool", "pspool", "ptpool", "sndpool", "gpool",
            "dinpool", "doutpool", "drive_f", "drive_b", "wr_b",
            "ident_f", "ident_b", "outf_d", "outu_d"))
    if True:
        xm_prev = None
        for t in range(t_steps):
            if t == 0:
                # xm_0 = drive
                xm = stpool.tile([128, NSH], F32, tag="xm")
                nc.vector.tensor_copy(xm[:], drive_f[:])
            else:
                ps = pspool.tile([128, NSH], F32, tag="ps")
                # drive contribution first: does not depend on the gather
                nc.tensor.matmul(ps[:], ident_b[:], drive_b[:], start=True, stop=False)
                for k in range(NKT):
                    nc.tensor.matmul(
                        ps[:],
                        g[:, 128 * k : 128 * (k + 1)],
                        wr_b[:, NSH * k : NSH * (k + 1)],
                        start=False,
                        stop=(k == NKT - 1),
                    )
                # xm = DECAY * xm_prev + (r + drive)   -- one fused DVE op
                xm = stpool.tile([128, NSH], F32, tag="xm")
                nc.vector.scalar_tensor_tensor(
                    xm[:], xm_prev[:], DECAY, ps[:], ALU.mult, ALU.add
                )

            # outputs: act (= xm) and firing rate f = relu(xm)
            nc.sync.dma_start(outu_d.ap()[:, t, :], xm[:])
            f32t = stpool.tile([128, NSH], F32, tag="f32")
            nc.scalar.activation(f32t[:], xm[:], AF.Relu)
            nc.sync.dma_start(outf_d.ap()[:, t, :], f32t[:])

            if t == t_steps - 1:
                break

            # transpose xm -> 4 tiles, relu+cast into bf16 send buffer
            sb = sndpool.tile([128, NSH], BF, tag="sb")
            pst = ptpool.tile([128, NSH], F32, tag="pst")
            for i in range(NSH // 128):
                nc.tensor.transpose(
                    pst[:, 128 * i : 128 * (i + 1)],
                    xm[:, 128 * i : 128 * (i + 1)],
                    ident_f[:],
                )
            nc.vector.tensor_scalar_max(sb[:], pst[:], 0.0)

            # all-gather the fT shards via DRAM bounce buffers
            snd_dram = dinpool.tile([128, NSH], BF, tag="snd")
            rcv_dram = doutpool.tile([NCORES, 128, NSH], BF, tag="rcv", addr_space="Shared")
            nc.sync.dma_start(snd_dram[:], sb[:])
            nc.gpsimd.collective_compute(
                "AllGather",
                ALU.bypass,
                ins=[snd_dram.opt()],
                outs=[rcv_dram.opt()],
                replica_groups=[list(range(NCORES))],
            )
            g = gpool.tile([128, N], BF, tag="g")
            nc.sync.dma_start(
                g.rearrange("p (r m) -> p r m", r=NCORES),
                rcv_dram.rearrange("r p m -> p r m"),
            )

            xm_prev = xm


_MODULE_CACHE = {}


def _get_module(t_steps=T):
    if t_steps not in _MODULE_CACHE:
        _MODULE_CACHE[t_steps] = build_module(t_steps)
    return _MODULE_CACHE[t_steps]


def make_in_maps(x, Wi, b0, Wr):
    x = np.ascontiguousarray(np.asarray(x, dtype=np.float32))
    Wi = np.asarray(Wi, dtype=np.float32)
    Wr = np.asarray(Wr, dtype=np.float32)
    b0 = np.asarray(b0, dtype=np.float32).reshape(1, N)
    in_maps = []
    for c in range(NCORES):
        sl = slice(c * NSH, (c + 1) * NSH)
        in_maps.append(
            {
                "x": x,
                "wi": np.ascontiguousarray(Wi[:, sl]),
                "wr": np.ascontiguousarray(Wr[:, sl]),
                "b0": np.ascontiguousarray(b0[:, sl]),
            }
        )
    return in_maps


def run(x, Wi, b0, Wr, t_steps=T, trace=False):
    nc = _get_module(t_steps)
    in_maps = make_in_maps(x, Wi, b0, Wr)
    res = run_bass_kernel_spmd(
        nc, in_maps, core_ids=list(range(NCORES)), trace=trace
    )
    outf = np.concatenate([res.results[c]["outf"] for c in range(NCORES)], axis=2)
    outu = np.concatenate([res.results[c]["outu"] for c in range(NCORES)], axis=2)
    return (outf, outu), res


def kernel(x, Wi, b0, Wr):
    trace = bool(int(os.environ.get("RNN_KERNEL_TRACE", "0")))
    (outf, outu), res = run(x, Wi, b0, Wr, trace=trace)
    if trace and res.exec_time_ns is not None:
        print(f"HW exec time: {res.exec_time_ns} ns")
    return outf, outu
